# revision 15
# baseline (speedup 1.0000x reference)
"""Trainium2 Bass kernel for windowed (local) causal self-attention.

Reference computation (per batch element, fp32):
    q = x @ Wq.T + bq ; k = x @ Wk.T + bk ; v = x @ Wv.T + bv
    per non-overlapping window of 256 tokens:
        attn = softmax(causal_mask(q k^T * HEAD_DIM**-0.5))
        out  = attn @ v
    o = out @ Wo.T + bo + x

Algebraic restructure (no head split in this module, softmax rows sum to 1):
    scores = q k^T = x M x^T + cq 1^T + 1 ck^T + bq.bk,  M  = Wq^T Wk
        cq = x (Wq^T bk)  [per-QUERY shift: cancels in softmax, dropped]
        ck = x (Wk^T bq)  [per-KEY: folded into the ACT exp bias]
    o = attn (x N) + (bv Wo^T + bo) + x,      N  = Wv^T Wo^T
so only TWO E x E projections remain on device (q' = x M and v' = x N);
M, N, ck and the constant output row are computed on the host in float64.
The residual + constant row are also added on the host.

Sharding: data-parallel over (batch, window): 64 window-blocks of 256
tokens -> 8 cores x 8 windows.  M, N replicated.

Per-core kernel strategy:
  - ALL matmul operands are bf16 (PE streams 1 col/cycle regardless of
    dtype, so bf16 matches fp32r rate at half the DMA/SBUF traffic and,
    critically, every LDWEIGHTS is a cheap 97ns FWL load -- no fp32-mode
    two-pass weight loads anywhere).  PSUM accumulation is fp32; the
    softmax chain (mask add, exp, row sums, reciprocal) stays fp32
    except the stored exp values (bf16, 0.4% rel err on attn weights).
  - scores are computed TRANSPOSED, sT[k, q] = x_k . q'_q, so no PE
    transposes of the attention matrix are needed: exp(sT) chunks serve
    directly as the stationary operand of out = attn @ v'.
  - causal block-sparsity: the kt=1 key block only serves queries q>=128,
    so its score matmuls/exp narrow to 128 columns and the (qt=0, kt=1)
    output/sum matmuls are skipped.
  - softmax row sums become N=2 matmuls (expT^T @ ones2) accumulated over
    k-chunks; normalization is folded into the ACT output evacuation as a
    per-partition scale (1/sum).
  - q'-projection is window-PAIRED (moving 512 tokens) to halve its
    instruction count and PE weight-load switches.  qT is split into
    per-eo tiles so the first score matmuls of a pair start as soon as
    their own chunk is evacuated (no whole-tile dependency stall).
  - v' is computed token-major between the score matmuls and the
    attention matmuls so the PE stays busy through the softmax chain.
  - ALL dram tensors are pre-arranged on the host into the exact SBUF
    tile layouts, so every DMA moves contiguous per-partition lines
    (1-16KB packets instead of 256B gather packets).  Boot: tiny consts,
    then pair-0 xT (2 contiguous halves), then M in 8 eo-chunks paced
    with tiny PE warmup matmuls; the first projection matmuls start as
    soon as (xT, M chunk 0) land.  N chunks and the next pair's xT are
    prefetched behind pair-0 compute; output stores (bf16, host upcasts
    and adds the residual) are deferred behind the next pair's loads.
"""
import sys

sys.path.insert(0, "/opt/trn_rl_repo")

import numpy as np
import ml_dtypes

import concourse.bass as bass
import concourse.bacc as bacc
import concourse.mybir as mybir
import concourse.tile as tile
from concourse.bass_utils import run_bass_kernel_spmd

F32 = mybir.dt.float32
BF16 = mybir.dt.bfloat16
NP_BF16 = ml_dtypes.bfloat16
AF = mybir.ActivationFunctionType

E = 1024          # embed dim
ET = E // 128     # e-tiles
W = 256           # window size
NW = 8            # windows per core
T = NW * W        # tokens per core
N_CORES = 8
SCALE = (E // 16) ** (-0.5)  # HEAD_DIM ** -0.5 = 0.125
NEG = -1.0e30
PW = 2 * W        # tokens per window pair


def build_nc(nw=NW):
    t_core = nw * W
    npair = nw // 2
    nc = bacc.Bacc("TRN2", target_bir_lowering=False, debug=False)

    # host-prearranged layouts (exactly the SBUF tile layouts):
    #   xt[pair][p][ei][t]  = x[pair*PW + t, ei*128 + p]
    #   m[p][eo][ei][c]     = M[ei*128 + p, eo*128 + c]
    #   n[p][ei][c]         = N[ei*128 + p, c]
    xt_d = nc.dram_tensor("xt", [npair, 128, ET, PW], BF16, kind="ExternalInput")
    m_d = nc.dram_tensor("m", [128, ET, ET, 128], BF16, kind="ExternalInput")
    n_d = nc.dram_tensor("n", [128, ET, E], BF16, kind="ExternalInput")
    # ck * SCALE laid out as one [128] column per 128-token chunk
    ckc_d = nc.dram_tensor("ckc", [128, 2 * nw], F32, kind="ExternalInput")
    o_d = nc.dram_tensor("o", [t_core, E], BF16, kind="ExternalOutput")

    # transposed causal mask constants applied to sT[k, q] = score(q, k).
    # kt=0: full [k, 0:256] triangle.  kt=1: only queries q>=128 are kept
    # downstream, stored in columns 0:128 (q = 128 + col).  (The bq.bk
    # score constant and the per-query cq row are dropped: uniform per-row
    # logit shifts cancel in softmax.)
    mask_np = np.full((2, 128, W), NEG, dtype=np.float32)
    k_idx = np.arange(128)[:, None]
    mask_np[0][k_idx <= np.arange(W)[None, :]] = 0.0
    mask_np[1][:, 0:128][k_idx <= np.arange(128)[None, :]] = 0.0
    # single [128, 512] row-contiguous image (2KB DMA lines)
    mask_d = nc.inline_tensor(
        np.ascontiguousarray(mask_np.transpose(1, 0, 2).reshape(128, 2 * W)), "mask"
    )

    with tile.TileContext(nc) as tc:
        with (
            tc.tile_pool(name="wp", bufs=1) as wp,
            tc.tile_pool(name="cp", bufs=1) as cp,
            tc.tile_pool(name="xtp", bufs=2) as xtp,
            tc.tile_pool(name="qtp", bufs=2 * ET) as qtp,
            tc.tile_pool(name="etp", bufs=2) as etp,
            tc.tile_pool(name="sp", bufs=4) as sp,
            tc.tile_pool(name="vp", bufs=4) as vp,
            tc.tile_pool(name="smp", bufs=8) as smp,
            tc.tile_pool(name="op", bufs=4) as op,
            tc.tile_pool(name="ps_qk", bufs=2, space=bass.MemorySpace.PSUM) as ps_qk,
            tc.tile_pool(name="ps_big", bufs=4, space=bass.MemorySpace.PSUM) as ps_big,
            tc.tile_pool(name="ps_sm", bufs=1, space=bass.MemorySpace.PSUM) as ps_sm,
            tc.tile_pool(name="ps_wm", bufs=1, space=bass.MemorySpace.PSUM) as ps_wm,
        ):
            # ---- resident constants ----
            masks = cp.tile([128, 2, W], F32, tag="mask")
            onec = cp.tile([128, 2], BF16, tag="onec")
            ckc = cp.tile([128, 2 * nw], F32, tag="ckc")
            # warm tile: DVE-memset (no DMA dependency) so the PE warmup
            # stream can start at engine-ready, flipping the HAM clock gate
            # to 8/8 before the first real matmul issues
            wsb = cp.tile([128, 256], BF16, tag="wsb")

            # ---- resident weights in tile layout ----
            msb = wp.tile([128, ET, ET, 128], BF16, tag="m", name="msb")
            nsb = wp.tile([128, ET, E], BF16, tag="n", name="nsb")

            def warm(moving=None):
                # a PE-busy matmul: 256 bf16 moving columns keeps the PE
                # activity monitor's busy-duty high through boot.  An
                # explicit `moving` slice paces the warmup on that DMA.
                wps = ps_wm.tile([128, 256], F32, tag="warm", name="warm")
                mv = wsb[:, 0:256] if moving is None else moving
                nc.tensor.matmul(
                    wps[:], wsb[:, 0:128], mv, start=True, stop=True
                )

            xT_next = None
            for p in range(npair):
                ptok0 = p * PW

                # ---- xT[p, ei, t] for the pair (host-prearranged) ----
                if p == 0:
                    nc.vector.memset(wsb[:], 1.0)
                    nc.vector.memset(onec[:], 1.0)
                    # upfront warmup matmuls start warming the HAM clock
                    # gate at engine-ready; the q' eo-group 0 matmuls then
                    # self-pace on the per-ei xT chunk arrivals and keep
                    # the busy-duty up until HAM flips to full clock
                    for _ in range(6):
                        warm()
                    # interleave: xT ei-chunk 0, M chunk 0 (so the first
                    # projection matmul starts ~1.5us after DMA start),
                    # then the rest of xT per-ei (group 0's accumulation
                    # consumes them in arrival order), then the remaining
                    # M eo-chunks.  Per-engine DMA queues are FIFO, so
                    # emission order is arrival order.
                    xT = xtp.tile([128, ET, PW], BF16, tag="xT")
                    nc.sync.dma_start(xT[:, 0:1, :], xt_d.ap()[0, :, 0:1, :])
                    nc.sync.dma_start(msb[:, 0], m_d.ap()[:, 0])
                    for h in range(1, ET):
                        nc.sync.dma_start(
                            xT[:, h : h + 1, :],
                            xt_d.ap()[0, :, h : h + 1, :],
                        )
                    for eo in range(1, ET):
                        nc.sync.dma_start(msb[:, eo], m_d.ap()[:, eo])
                    nc.sync.dma_start(
                        masks[:, :, :],
                        mask_d.ap().rearrange("p (k w) -> p k w", k=2),
                    )
                else:
                    xT = xT_next

                # ---- q' projection for the pair -> per-eo q'T [128, 512] ----
                qts = []
                for eo in range(ET):
                    pp = ps_big.tile([128, PW], F32, tag="big")
                    for ei in range(ET):
                        nc.tensor.matmul(
                            pp[:],
                            msb[:, eo, ei, :],
                            xT[:, ei, :],
                            start=(ei == 0),
                            stop=(ei == ET - 1),
                        )
                    qt = qtp.tile([128, PW], BF16, tag="qT")
                    nc.scalar.copy(qt[:], pp[:])
                    qts.append(qt)

                if p == 0:
                    # N chunks + ckc slot in behind the pair-0 q' matmuls
                    for eq in range(0, ET, 2):
                        nc.sync.dma_start(
                            nsb[:, eq : eq + 2, :], n_d.ap()[:, eq : eq + 2, :]
                        )
                    nc.sync.dma_start(ckc[:], ckc_d.ap())

                # prefetch next pair's xT behind this pair's compute
                if p + 1 < npair:
                    xT_next = xtp.tile([128, ET, PW], BF16, tag="xT")
                    nc.sync.dma_start(xT_next[:, :, :], xt_d.ap()[p + 1])

                for wi in range(2):
                    w = 2 * p + wi
                    tok0 = w * W
                    wt0 = wi * W  # token offset inside the pair tiles

                    # ---- transposed scores sT[k, q] + softmax ----
                    # causal: the kt=1 key block only serves queries q>=128;
                    # its block is computed 128 columns wide (q = 128+col).
                    expT = etp.tile([128, 2, W], BF16, tag="expT")
                    for kt in range(2):
                        qw = W if kt == 0 else 128
                        q0 = wt0 + kt * 128  # first query column needed
                        sc = ps_qk.tile([128, W], F32, tag="qk")
                        for ei in range(ET):
                            nc.tensor.matmul(
                                sc[:, 0:qw],
                                xT[:, ei, wt0 + kt * 128 : wt0 + (kt + 1) * 128],
                                qts[ei][:, q0 : q0 + qw],
                                start=(ei == 0),
                                stop=(ei == ET - 1),
                            )
                        s_sb = sp.tile([128, W], F32, tag="s")
                        nc.vector.tensor_add(
                            s_sb[:, 0:qw], sc[:, 0:qw], masks[:, kt, 0:qw]
                        )
                        # exp(SCALE*s + SCALE*ck[k]): per-key bias via ACT
                        nc.scalar.activation(
                            expT[:, kt, kt * 128 : kt * 128 + qw],
                            s_sb[:, 0:qw],
                            AF.Exp,
                            scale=SCALE,
                            bias=ckc[:, 2 * w + kt : 2 * w + kt + 1],
                        )

                    # ---- v' projection (token-major), fills PE during softmax ----
                    v_w = [
                        vp.tile([128, E], BF16, tag="v", name=f"v{kt}")
                        for kt in range(2)
                    ]
                    for kt in range(2):
                        for eoh in range(2):
                            pv = ps_big.tile([128, 512], F32, tag="big")
                            for ei in range(ET):
                                nc.tensor.matmul(
                                    pv[:],
                                    xT[:, ei, wt0 + kt * 128 : wt0 + (kt + 1) * 128],
                                    nsb[:, ei, eoh * 512 : (eoh + 1) * 512],
                                    start=(ei == 0),
                                    stop=(ei == ET - 1),
                                )
                            nc.vector.tensor_copy(
                                v_w[kt][:, eoh * 512 : (eoh + 1) * 512], pv[:]
                            )

                    # ---- softmax row sums (over k = partitions) ----
                    # qt=0 queries only attend to kt=0 keys (causal)
                    recs = []
                    for qt in range(2):
                        kts = (0,) if qt == 0 else (0, 1)
                        sm = ps_sm.tile([128, 2], F32, tag="sum")
                        for kt in kts:
                            nc.tensor.matmul(
                                sm[:],
                                expT[:, kt, qt * 128 : (qt + 1) * 128],
                                onec[:],
                                start=(kt == kts[0]),
                                stop=(kt == kts[-1]),
                            )
                        rec = smp.tile([128, 1], F32, tag="rec")
                        nc.vector.reciprocal(rec[:], sm[:, 0:1])
                        recs.append(rec)

                    # ---- out = attn @ v' (token-major), normalize in evac ----
                    for qt in range(2):
                        kts = (0,) if qt == 0 else (0, 1)
                        o_sb = op.tile([128, E], BF16, tag="o")
                        for eoh in range(2):
                            po = ps_big.tile([128, 512], F32, tag="big")
                            for kt in kts:
                                nc.tensor.matmul(
                                    po[:],
                                    expT[:, kt, qt * 128 : (qt + 1) * 128],
                                    v_w[kt][:, eoh * 512 : (eoh + 1) * 512],
                                    start=(kt == kts[0]),
                                    stop=(kt == kts[-1]),
                                )
                            # normalize-evacuate on ACT; for the final
                            # window alternate ACT/DVE per half so both
                            # engines drain the tail concurrently
                            if w < nw - 1 or eoh == 0:
                                nc.scalar.activation(
                                    o_sb[:, eoh * 512 : (eoh + 1) * 512],
                                    po[:],
                                    AF.Copy,
                                    scale=recs[qt][:],
                                )
                            else:
                                nc.vector.tensor_scalar_mul(
                                    o_sb[:, eoh * 512 : (eoh + 1) * 512],
                                    po[:],
                                    recs[qt][:],
                                )
                        # one store per qt (each DMA instruction costs
                        # ~600ns of sync-engine issue, so fewer + larger
                        # wins even on the final window's drain).  Stores
                        # are emitted after the next pair's prefetch in
                        # sync-queue order, so they never head-of-line-
                        # block it.
                        nc.sync.dma_start(
                            o_d.ap()[tok0 + qt * 128 : tok0 + (qt + 1) * 128, :],
                            o_sb[:],
                        )

    nc.compile()
    return nc


_NC_CACHE = {}


def _get_nc(nw=NW):
    if nw not in _NC_CACHE:
        _NC_CACHE[nw] = build_nc(nw)
    return _NC_CACHE[nw]


def prepare(x, Wq, bq, Wk, bk, Wv, bv, Wo, bo):
    """Host-side precompute: per-core input maps + host residual terms."""
    x = np.asarray(x, dtype=np.float32)
    B, S, _ = x.shape
    x_flat = np.ascontiguousarray(x.reshape(B * S, E))
    t_core = B * S // N_CORES
    assert t_core == T
    npair = T // PW

    f64 = np.float64
    Wq64, Wk64 = np.asarray(Wq, f64), np.asarray(Wk, f64)
    Wv64, Wo64 = np.asarray(Wv, f64), np.asarray(Wo, f64)
    bq64 = np.asarray(bq, f64)
    bv64, bo64 = np.asarray(bv, f64), np.asarray(bo, f64)

    M = (Wq64.T @ Wk64).astype(np.float32)
    N = (Wv64.T @ Wo64.T).astype(np.float32)
    # tile layouts: m[p][eo][ei][c] = M[ei*128+p, eo*128+c]; n[p][ei][c]
    m_t = np.ascontiguousarray(
        M.reshape(ET, 128, ET, 128).transpose(1, 2, 0, 3).astype(NP_BF16)
    )
    n_t = np.ascontiguousarray(
        N.reshape(ET, 128, E).transpose(1, 0, 2).astype(NP_BF16)
    )
    ck = (x_flat.astype(f64) @ (Wk64.T @ bq64)) * SCALE  # [T_total]
    orow = (bv64 @ Wo64.T + bo64).astype(np.float32)  # [E]

    common = {"m": m_t, "n": n_t}
    in_maps = []
    for i in range(N_CORES):
        xc = x_flat[i * t_core : (i + 1) * t_core]
        # xt[pair][p][ei][t] = x[pair*PW + t, ei*128 + p]
        xt = np.ascontiguousarray(
            xc.reshape(npair, PW, ET, 128).transpose(0, 3, 2, 1).astype(NP_BF16)
        )
        in_maps.append(
            {
                "xt": xt,
                # ck columns: [128, 2*nw], one column per 128-token chunk
                "ckc": np.ascontiguousarray(
                    ck[i * t_core : (i + 1) * t_core]
                    .astype(np.float32)
                    .reshape(2 * NW, 128)
                    .T
                ),
                **common,
            }
        )
    return in_maps, orow, x_flat, (B, S)


def kernel(x, Wq, bq, Wk, bk, Wv, bv, Wo, bo):
    in_maps, orow, x_flat, (B, S) = prepare(x, Wq, bq, Wk, bk, Wv, bv, Wo, bo)
    nc = _get_nc()
    res = run_bass_kernel_spmd(nc, in_maps, core_ids=list(range(N_CORES)))
    out = np.concatenate(
        [np.asarray(res.results[i]["o"]).astype(np.float32) for i in range(N_CORES)],
        axis=0,
    )
    out += orow[None, :]
    out += x_flat
    return out.reshape(B, S, E).astype(np.float32)


# revision 19
# speedup vs baseline: 1.0182x; 1.0182x over previous
"""Trainium2 Bass kernel for windowed (local) causal self-attention.

Reference computation (per batch element, fp32):
    q = x @ Wq.T + bq ; k = x @ Wk.T + bk ; v = x @ Wv.T + bv
    per non-overlapping window of 256 tokens:
        attn = softmax(causal_mask(q k^T * HEAD_DIM**-0.5))
        out  = attn @ v
    o = out @ Wo.T + bo + x

Algebraic restructure (no head split in this module, softmax rows sum to 1):
    scores = q k^T = x M x^T + cq 1^T + 1 ck^T + bq.bk,  M  = Wq^T Wk
        cq = x (Wq^T bk)  [per-QUERY shift: cancels in softmax, dropped]
        ck = x (Wk^T bq)  [per-KEY: folded into the ACT exp bias]
    o = attn (x N) + (bv Wo^T + bo) + x,      N  = Wv^T Wo^T
so only TWO E x E projections remain on device (q' = x M and v' = x N);
M, N, ck and the constant output row are computed on the host in float64.
The residual + constant row are also added on the host.

Sharding: data-parallel over (batch, window): 64 window-blocks of 256
tokens -> 8 cores x 8 windows.  M, N replicated.

Per-core kernel strategy:
  - ALL matmul operands are bf16 (PE streams 1 col/cycle regardless of
    dtype, so bf16 matches fp32r rate at half the DMA/SBUF traffic and,
    critically, every LDWEIGHTS is a cheap 97ns FWL load -- no fp32-mode
    two-pass weight loads anywhere).  PSUM accumulation is fp32; the
    softmax chain (mask add, exp, row sums, reciprocal) stays fp32
    except the stored exp values (bf16, 0.4% rel err on attn weights).
  - scores are computed TRANSPOSED, sT[k, q] = x_k . q'_q, so no PE
    transposes of the attention matrix are needed: exp(sT) chunks serve
    directly as the stationary operand of out = attn @ v'.
  - causal block-sparsity: the kt=1 key block only serves queries q>=128,
    so its score matmuls/exp narrow to 128 columns and the (qt=0, kt=1)
    output/sum matmuls are skipped.
  - softmax row sums become N=2 matmuls (expT^T @ ones2) accumulated over
    k-chunks; normalization is folded into the ACT output evacuation as a
    per-partition scale (1/sum).
  - q'-projection is window-PAIRED (moving 512 tokens) to halve its
    instruction count and PE weight-load switches.  qT is split into
    per-eo tiles so the first score matmuls of a pair start as soon as
    their own chunk is evacuated (no whole-tile dependency stall).
  - v' is computed token-major between the score matmuls and the
    attention matmuls so the PE stays busy through the softmax chain.
  - ALL dram tensors are pre-arranged on the host into the exact SBUF
    tile layouts, so every DMA moves contiguous per-partition lines
    (1-16KB packets instead of 256B gather packets).  Boot: tiny consts,
    then pair-0 xT (2 contiguous halves), then M in 8 eo-chunks paced
    with tiny PE warmup matmuls; the first projection matmuls start as
    soon as (xT, M chunk 0) land.  N chunks and the next pair's xT are
    prefetched behind pair-0 compute; output stores (bf16, host upcasts
    and adds the residual) are deferred behind the next pair's loads.
"""
import sys

sys.path.insert(0, "/opt/trn_rl_repo")

import numpy as np
import ml_dtypes

import concourse.bass as bass
import concourse.bacc as bacc
import concourse.mybir as mybir
import concourse.tile as tile
from concourse.bass_utils import run_bass_kernel_spmd

F32 = mybir.dt.float32
BF16 = mybir.dt.bfloat16
NP_BF16 = ml_dtypes.bfloat16
AF = mybir.ActivationFunctionType

E = 1024          # embed dim
ET = E // 128     # e-tiles
W = 256           # window size
NW = 8            # windows per core
T = NW * W        # tokens per core
N_CORES = 8
SCALE = (E // 16) ** (-0.5)  # HEAD_DIM ** -0.5 = 0.125
NEG = -1.0e30
PW = 2 * W        # tokens per window pair


def build_nc(nw=NW):
    t_core = nw * W
    npair = nw // 2
    nc = bacc.Bacc("TRN2", target_bir_lowering=False, debug=False)

    # host-prearranged layouts (exactly the SBUF tile layouts):
    #   xt[pair][p][ei][t]  = x[pair*PW + t, ei*128 + p]
    #   m[p][eo][ei][c]     = M[ei*128 + p, eo*128 + c]
    #   n[p][ei][c]         = N[ei*128 + p, c]
    xt_d = nc.dram_tensor("xt", [npair, 128, ET, PW], BF16, kind="ExternalInput")
    m_d = nc.dram_tensor("m", [128, ET, ET, 128], BF16, kind="ExternalInput")
    n_d = nc.dram_tensor("n", [128, ET, E], BF16, kind="ExternalInput")
    # ck * SCALE laid out as one [128] column per 128-token chunk
    ckc_d = nc.dram_tensor("ckc", [128, 2 * nw], F32, kind="ExternalInput")
    o_d = nc.dram_tensor("o", [t_core, E], BF16, kind="ExternalOutput")

    # transposed causal mask constants applied to sT[k, q] = score(q, k).
    # kt=0: full [k, 0:256] triangle.  kt=1: only queries q>=128 are kept
    # downstream, stored in columns 0:128 (q = 128 + col).  (The bq.bk
    # score constant and the per-query cq row are dropped: uniform per-row
    # logit shifts cancel in softmax.)
    mask_np = np.full((2, 128, W), NEG, dtype=np.float32)
    k_idx = np.arange(128)[:, None]
    mask_np[0][k_idx <= np.arange(W)[None, :]] = 0.0
    mask_np[1][:, 0:128][k_idx <= np.arange(128)[None, :]] = 0.0
    # single [128, 512] row-contiguous image (2KB DMA lines)
    mask_d = nc.inline_tensor(
        np.ascontiguousarray(mask_np.transpose(1, 0, 2).reshape(128, 2 * W)), "mask"
    )

    with tile.TileContext(nc) as tc:
        with (
            tc.tile_pool(name="wp", bufs=1) as wp,
            tc.tile_pool(name="cp", bufs=1) as cp,
            tc.tile_pool(name="xtp", bufs=2) as xtp,
            tc.tile_pool(name="qtp", bufs=2 * ET) as qtp,
            tc.tile_pool(name="etp", bufs=2) as etp,
            tc.tile_pool(name="sp", bufs=4) as sp,
            tc.tile_pool(name="vp", bufs=4) as vp,
            tc.tile_pool(name="smp", bufs=8) as smp,
            tc.tile_pool(name="op", bufs=4) as op,
            tc.tile_pool(name="ps_qk", bufs=2, space=bass.MemorySpace.PSUM) as ps_qk,
            tc.tile_pool(name="ps_big", bufs=5, space=bass.MemorySpace.PSUM) as ps_big,
            tc.tile_pool(name="ps_sm", bufs=1, space=bass.MemorySpace.PSUM) as ps_sm,
        ):
            # ---- resident constants ----
            masks = cp.tile([128, 2, W], F32, tag="mask")
            onec = cp.tile([128, 2], BF16, tag="onec")
            ckc = cp.tile([128, 2 * nw], F32, tag="ckc")
            # warm tile: DVE-memset (no DMA dependency) so the PE warmup
            # stream can start at engine-ready, flipping the HAM clock gate
            # to 8/8 before the first real matmul issues
            wsb = cp.tile([128, 256], BF16, tag="wsb")

            # ---- resident weights in tile layout ----
            msb = wp.tile([128, ET, ET, 128], BF16, tag="m", name="msb")
            nsb = wp.tile([128, ET, E], BF16, tag="n", name="nsb")

            def warm():
                # a PE-busy matmul: 256 bf16 moving columns keeps the PE
                # activity monitor's busy-duty high through boot.  Uses
                # the (boot-idle) score bank so ps_big gets 5 buffers.
                wps = ps_qk.tile([128, W], F32, tag="qk", name="warm")
                nc.tensor.matmul(
                    wps[:], wsb[:, 0:128], wsb[:, 0:256], start=True, stop=True
                )

            xT_next = None
            for p in range(npair):
                ptok0 = p * PW

                # ---- xT[p, ei, t] for the pair (host-prearranged) ----
                if p == 0:
                    nc.vector.memset(wsb[:], 1.0)
                    nc.vector.memset(onec[:], 1.0)
                    # upfront warmup matmuls start warming the HAM clock
                    # gate at engine-ready and bridge the DMA wait until
                    # the q' eo-group 0 matmuls take over (they self-pace
                    # on the per-2-ei xT chunk arrivals)
                    for _ in range(9):
                        warm()
                    # interleave: xT chunk 0, M chunk 0 (so the first
                    # projection matmuls start ~3us after DMA start),
                    # then the rest of xT (group 0's accumulation consumes
                    # them in arrival order), then the remaining M chunks.
                    # Per-engine DMA queues are FIFO, so emission order is
                    # arrival order.
                    xT = xtp.tile([128, ET, PW], BF16, tag="xT")
                    nc.sync.dma_start(xT[:, 0:2, :], xt_d.ap()[0, :, 0:2, :])
                    nc.sync.dma_start(msb[:, 0], m_d.ap()[:, 0])
                    for h in range(1, 4):
                        nc.sync.dma_start(
                            xT[:, 2 * h : 2 * h + 2, :],
                            xt_d.ap()[0, :, 2 * h : 2 * h + 2, :],
                        )
                    for eo in range(1, ET):
                        nc.sync.dma_start(msb[:, eo], m_d.ap()[:, eo])
                    nc.sync.dma_start(
                        masks[:, :, :],
                        mask_d.ap().rearrange("p (k w) -> p k w", k=2),
                    )
                else:
                    xT = xT_next

                # ---- q' projection for the pair -> per-eo q'T [128, 512] ----
                qts = []
                for eo in range(ET):
                    pp = ps_big.tile([128, PW], F32, tag="big")
                    for ei in range(ET):
                        nc.tensor.matmul(
                            pp[:],
                            msb[:, eo, ei, :],
                            xT[:, ei, :],
                            start=(ei == 0),
                            stop=(ei == ET - 1),
                        )
                    qt = qtp.tile([128, PW], BF16, tag="qT")
                    nc.scalar.copy(qt[:], pp[:])
                    qts.append(qt)

                if p == 0:
                    # N chunks + ckc slot in behind the pair-0 q' matmuls
                    for eq in range(0, ET, 2):
                        nc.sync.dma_start(
                            nsb[:, eq : eq + 2, :], n_d.ap()[:, eq : eq + 2, :]
                        )
                    nc.sync.dma_start(ckc[:], ckc_d.ap())

                # prefetch next pair's xT behind this pair's compute
                if p + 1 < npair:
                    xT_next = xtp.tile([128, ET, PW], BF16, tag="xT")
                    nc.sync.dma_start(xT_next[:, :, :], xt_d.ap()[p + 1])

                for wi in range(2):
                    w = 2 * p + wi
                    tok0 = w * W
                    wt0 = wi * W  # token offset inside the pair tiles

                    # ---- transposed scores sT[k, q] + softmax ----
                    # causal: the kt=1 key block only serves queries q>=128;
                    # its block is computed 128 columns wide (q = 128+col).
                    expT = etp.tile([128, 2, W], BF16, tag="expT")
                    for kt in range(2):
                        qw = W if kt == 0 else 128
                        q0 = wt0 + kt * 128  # first query column needed
                        sc = ps_qk.tile([128, W], F32, tag="qk")
                        for ei in range(ET):
                            nc.tensor.matmul(
                                sc[:, 0:qw],
                                xT[:, ei, wt0 + kt * 128 : wt0 + (kt + 1) * 128],
                                qts[ei][:, q0 : q0 + qw],
                                start=(ei == 0),
                                stop=(ei == ET - 1),
                            )
                        s_sb = sp.tile([128, W], F32, tag="s")
                        nc.vector.tensor_add(
                            s_sb[:, 0:qw], sc[:, 0:qw], masks[:, kt, 0:qw]
                        )
                        # exp(SCALE*s + SCALE*ck[k]): per-key bias via ACT
                        nc.scalar.activation(
                            expT[:, kt, kt * 128 : kt * 128 + qw],
                            s_sb[:, 0:qw],
                            AF.Exp,
                            scale=SCALE,
                            bias=ckc[:, 2 * w + kt : 2 * w + kt + 1],
                        )

                    # ---- v' projection (token-major), fills PE during softmax ----
                    v_w = [
                        vp.tile([128, E], BF16, tag="v", name=f"v{kt}")
                        for kt in range(2)
                    ]
                    for kt in range(2):
                        for eoh in range(2):
                            pv = ps_big.tile([128, 512], F32, tag="big")
                            for ei in range(ET):
                                nc.tensor.matmul(
                                    pv[:],
                                    xT[:, ei, wt0 + kt * 128 : wt0 + (kt + 1) * 128],
                                    nsb[:, ei, eoh * 512 : (eoh + 1) * 512],
                                    start=(ei == 0),
                                    stop=(ei == ET - 1),
                                )
                            nc.vector.tensor_copy(
                                v_w[kt][:, eoh * 512 : (eoh + 1) * 512], pv[:]
                            )

                    # ---- softmax row sums (over k = partitions) ----
                    # qt=0 queries only attend to kt=0 keys (causal)
                    recs = []
                    for qt in range(2):
                        kts = (0,) if qt == 0 else (0, 1)
                        sm = ps_sm.tile([128, 2], F32, tag="sum")
                        for kt in kts:
                            nc.tensor.matmul(
                                sm[:],
                                expT[:, kt, qt * 128 : (qt + 1) * 128],
                                onec[:],
                                start=(kt == kts[0]),
                                stop=(kt == kts[-1]),
                            )
                        rec = smp.tile([128, 1], F32, tag="rec")
                        nc.vector.reciprocal(rec[:], sm[:, 0:1])
                        recs.append(rec)

                    # ---- out = attn @ v' (token-major), normalize in evac ----
                    for qt in range(2):
                        kts = (0,) if qt == 0 else (0, 1)
                        o_sb = op.tile([128, E], BF16, tag="o")
                        for eoh in range(2):
                            po = ps_big.tile([128, 512], F32, tag="big")
                            for kt in kts:
                                nc.tensor.matmul(
                                    po[:],
                                    expT[:, kt, qt * 128 : (qt + 1) * 128],
                                    v_w[kt][:, eoh * 512 : (eoh + 1) * 512],
                                    start=(kt == kts[0]),
                                    stop=(kt == kts[-1]),
                                )
                            # normalize-evacuate on ACT; for the final
                            # window alternate ACT/DVE per half so both
                            # engines drain the tail concurrently
                            if w < nw - 1 or eoh == 0:
                                nc.scalar.activation(
                                    o_sb[:, eoh * 512 : (eoh + 1) * 512],
                                    po[:],
                                    AF.Copy,
                                    scale=recs[qt][:],
                                )
                            else:
                                nc.vector.tensor_scalar_mul(
                                    o_sb[:, eoh * 512 : (eoh + 1) * 512],
                                    po[:],
                                    recs[qt][:],
                                )
                        # one store per qt (each DMA instruction costs
                        # ~600ns of sync-engine issue, so fewer + larger
                        # wins even on the final window's drain).  Stores
                        # are emitted after the next pair's prefetch in
                        # sync-queue order, so they never head-of-line-
                        # block it.
                        nc.sync.dma_start(
                            o_d.ap()[tok0 + qt * 128 : tok0 + (qt + 1) * 128, :],
                            o_sb[:],
                        )

    nc.compile()
    return nc


_NC_CACHE = {}


def _get_nc(nw=NW):
    if nw not in _NC_CACHE:
        _NC_CACHE[nw] = build_nc(nw)
    return _NC_CACHE[nw]


def prepare(x, Wq, bq, Wk, bk, Wv, bv, Wo, bo):
    """Host-side precompute: per-core input maps + host residual terms."""
    x = np.asarray(x, dtype=np.float32)
    B, S, _ = x.shape
    x_flat = np.ascontiguousarray(x.reshape(B * S, E))
    t_core = B * S // N_CORES
    assert t_core == T
    npair = T // PW

    f64 = np.float64
    Wq64, Wk64 = np.asarray(Wq, f64), np.asarray(Wk, f64)
    Wv64, Wo64 = np.asarray(Wv, f64), np.asarray(Wo, f64)
    bq64 = np.asarray(bq, f64)
    bv64, bo64 = np.asarray(bv, f64), np.asarray(bo, f64)

    M = (Wq64.T @ Wk64).astype(np.float32)
    N = (Wv64.T @ Wo64.T).astype(np.float32)
    # tile layouts: m[p][eo][ei][c] = M[ei*128+p, eo*128+c]; n[p][ei][c]
    m_t = np.ascontiguousarray(
        M.reshape(ET, 128, ET, 128).transpose(1, 2, 0, 3).astype(NP_BF16)
    )
    n_t = np.ascontiguousarray(
        N.reshape(ET, 128, E).transpose(1, 0, 2).astype(NP_BF16)
    )
    ck = (x_flat.astype(f64) @ (Wk64.T @ bq64)) * SCALE  # [T_total]
    orow = (bv64 @ Wo64.T + bo64).astype(np.float32)  # [E]

    common = {"m": m_t, "n": n_t}
    in_maps = []
    for i in range(N_CORES):
        xc = x_flat[i * t_core : (i + 1) * t_core]
        # xt[pair][p][ei][t] = x[pair*PW + t, ei*128 + p]
        xt = np.ascontiguousarray(
            xc.reshape(npair, PW, ET, 128).transpose(0, 3, 2, 1).astype(NP_BF16)
        )
        in_maps.append(
            {
                "xt": xt,
                # ck columns: [128, 2*nw], one column per 128-token chunk
                "ckc": np.ascontiguousarray(
                    ck[i * t_core : (i + 1) * t_core]
                    .astype(np.float32)
                    .reshape(2 * NW, 128)
                    .T
                ),
                **common,
            }
        )
    return in_maps, orow, x_flat, (B, S)


def kernel(x, Wq, bq, Wk, bk, Wv, bv, Wo, bo):
    in_maps, orow, x_flat, (B, S) = prepare(x, Wq, bq, Wk, bk, Wv, bv, Wo, bo)
    nc = _get_nc()
    res = run_bass_kernel_spmd(nc, in_maps, core_ids=list(range(N_CORES)))
    out = np.concatenate(
        [np.asarray(res.results[i]["o"]).astype(np.float32) for i in range(N_CORES)],
        axis=0,
    )
    out += orow[None, :]
    out += x_flat
    return out.reshape(B, S, E).astype(np.float32)


# revision 20
# speedup vs baseline: 1.0469x; 1.0283x over previous
"""Trainium2 Bass kernel for windowed (local) causal self-attention.

Reference computation (per batch element, fp32):
    q = x @ Wq.T + bq ; k = x @ Wk.T + bk ; v = x @ Wv.T + bv
    per non-overlapping window of 256 tokens:
        attn = softmax(causal_mask(q k^T * HEAD_DIM**-0.5))
        out  = attn @ v
    o = out @ Wo.T + bo + x

Algebraic restructure (no head split in this module, softmax rows sum to 1):
    scores = q k^T = x M x^T + cq 1^T + 1 ck^T + bq.bk,  M  = Wq^T Wk
        cq = x (Wq^T bk)  [per-QUERY shift: cancels in softmax, dropped]
        ck = x (Wk^T bq)  [per-KEY: folded into the ACT exp bias]
    o = attn (x N) + (bv Wo^T + bo) + x,      N  = Wv^T Wo^T
so only TWO E x E projections remain on device (q' = x M and v' = x N);
M, N, ck and the constant output row are computed on the host in float64.
The residual + constant row are also added on the host.

Sharding: data-parallel over (batch, window): 64 window-blocks of 256
tokens -> 8 cores x 8 windows.  M, N replicated.

Per-core kernel strategy:
  - ALL matmul operands are bf16 (PE streams 1 col/cycle regardless of
    dtype, so bf16 matches fp32r rate at half the DMA/SBUF traffic and,
    critically, every LDWEIGHTS is a cheap 97ns FWL load -- no fp32-mode
    two-pass weight loads anywhere).  PSUM accumulation is fp32; the
    softmax chain (mask add, exp, row sums, reciprocal) stays fp32
    except the stored exp values (bf16, 0.4% rel err on attn weights).
  - scores are computed TRANSPOSED, sT[k, q] = x_k . q'_q, so no PE
    transposes of the attention matrix are needed: exp(sT) chunks serve
    directly as the stationary operand of out = attn @ v'.
  - causal block-sparsity: the kt=1 key block only serves queries q>=128,
    so its score matmuls/exp narrow to 128 columns and the (qt=0, kt=1)
    output/sum matmuls are skipped.
  - softmax row sums become N=2 matmuls (expT^T @ ones2) accumulated over
    k-chunks; normalization is folded into the ACT output evacuation as a
    per-partition scale (1/sum).
  - q'-projection is window-PAIRED (moving 512 tokens) to halve its
    instruction count and PE weight-load switches.  qT is split into
    per-eo tiles so the first score matmuls of a pair start as soon as
    their own chunk is evacuated (no whole-tile dependency stall).
  - v' is computed token-major between the score matmuls and the
    attention matmuls so the PE stays busy through the softmax chain.
  - ALL dram tensors are pre-arranged on the host into the exact SBUF
    tile layouts, so every DMA moves contiguous per-partition lines
    (1-16KB packets instead of 256B gather packets).  Boot: tiny consts,
    then pair-0 xT (2 contiguous halves), then M in 8 eo-chunks paced
    with tiny PE warmup matmuls; the first projection matmuls start as
    soon as (xT, M chunk 0) land.  N chunks and the next pair's xT are
    prefetched behind pair-0 compute; output stores (bf16, host upcasts
    and adds the residual) are deferred behind the next pair's loads.
"""
import sys

sys.path.insert(0, "/opt/trn_rl_repo")

import numpy as np
import ml_dtypes

import concourse.bass as bass
import concourse.bacc as bacc
import concourse.mybir as mybir
import concourse.tile as tile
from concourse.bass_utils import run_bass_kernel_spmd

F32 = mybir.dt.float32
BF16 = mybir.dt.bfloat16
NP_BF16 = ml_dtypes.bfloat16
AF = mybir.ActivationFunctionType

E = 1024          # embed dim
ET = E // 128     # e-tiles
W = 256           # window size
NW = 8            # windows per core
T = NW * W        # tokens per core
N_CORES = 8
SCALE = (E // 16) ** (-0.5)  # HEAD_DIM ** -0.5 = 0.125
NEG = -1.0e30
PW = 2 * W        # tokens per window pair


def build_nc(nw=NW):
    t_core = nw * W
    npair = nw // 2
    nc = bacc.Bacc("TRN2", target_bir_lowering=False, debug=False)

    # host-prearranged layouts (exactly the SBUF tile layouts):
    #   xt[pair][p][ei][t]  = x[pair*PW + t, ei*128 + p]
    #   m[p][eo][ei][c]     = M[ei*128 + p, eo*128 + c]
    #   n[p][ei][c]         = N[ei*128 + p, c]
    xt_d = nc.dram_tensor("xt", [npair, 128, ET, PW], BF16, kind="ExternalInput")
    m_d = nc.dram_tensor("m", [128, ET, ET, 128], BF16, kind="ExternalInput")
    n_d = nc.dram_tensor("n", [128, ET, E], BF16, kind="ExternalInput")
    # ck * SCALE laid out as one [128] column per 128-token chunk
    ckc_d = nc.dram_tensor("ckc", [128, 2 * nw], F32, kind="ExternalInput")
    o_d = nc.dram_tensor("o", [t_core, E], BF16, kind="ExternalOutput")

    # transposed causal mask constants applied to sT[k, q] = score(q, k).
    # kt=0: full [k, 0:256] triangle.  kt=1: only queries q>=128 are kept
    # downstream, stored in columns 0:128 (q = 128 + col).  (The bq.bk
    # score constant and the per-query cq row are dropped: uniform per-row
    # logit shifts cancel in softmax.)
    mask_np = np.full((2, 128, W), NEG, dtype=np.float32)
    k_idx = np.arange(128)[:, None]
    mask_np[0][k_idx <= np.arange(W)[None, :]] = 0.0
    mask_np[1][:, 0:128][k_idx <= np.arange(128)[None, :]] = 0.0
    # single [128, 512] row-contiguous image (2KB DMA lines)
    mask_d = nc.inline_tensor(
        np.ascontiguousarray(mask_np.transpose(1, 0, 2).reshape(128, 2 * W)), "mask"
    )

    with tile.TileContext(nc) as tc:
        with (
            tc.tile_pool(name="wp", bufs=1) as wp,
            tc.tile_pool(name="cp", bufs=1) as cp,
            tc.tile_pool(name="xtp", bufs=2) as xtp,
            tc.tile_pool(name="qtp", bufs=2 * ET) as qtp,
            tc.tile_pool(name="etp", bufs=2) as etp,
            tc.tile_pool(name="sp", bufs=4) as sp,
            tc.tile_pool(name="vp", bufs=4) as vp,
            tc.tile_pool(name="smp", bufs=8) as smp,
            tc.tile_pool(name="op", bufs=4) as op,
            tc.tile_pool(name="ps_qk", bufs=2, space=bass.MemorySpace.PSUM) as ps_qk,
            tc.tile_pool(name="ps_big", bufs=5, space=bass.MemorySpace.PSUM) as ps_big,
            tc.tile_pool(name="ps_sm", bufs=1, space=bass.MemorySpace.PSUM) as ps_sm,
        ):
            # ---- resident constants ----
            masks = cp.tile([128, 2, W], F32, tag="mask")
            onec = cp.tile([128, 2], BF16, tag="onec")
            ckc = cp.tile([128, 2 * nw], F32, tag="ckc")
            # warm tile: DVE-memset (no DMA dependency) so the PE warmup
            # stream can start at engine-ready, flipping the HAM clock gate
            # to 8/8 before the first real matmul issues
            wsb = cp.tile([128, 512], BF16, tag="wsb")

            # ---- resident weights in tile layout ----
            msb = wp.tile([128, ET, ET, 128], BF16, tag="m", name="msb")
            nsb = wp.tile([128, ET, E], BF16, tag="n", name="nsb")

            def warm():
                # a PE-busy matmul: 256 bf16 moving columns keeps the PE
                # activity monitor's busy-duty high through boot.  Uses
                # the (boot-idle) score bank so ps_big gets 5 buffers.
                wps = ps_big.tile([128, 512], F32, tag="big", name="warm")
                nc.tensor.matmul(
                    wps[:], wsb[:, 0:128], wsb[:], start=True, stop=True
                )

            xT_next = None
            for p in range(npair):
                ptok0 = p * PW

                # ---- xT[p, ei, t] for the pair (host-prearranged) ----
                if p == 0:
                    nc.vector.memset(wsb[:], 1.0)
                    nc.vector.memset(onec[:], 1.0)
                    # upfront warmup matmuls start warming the HAM clock
                    # gate at engine-ready and bridge the DMA wait until
                    # the q' eo-group 0 matmuls take over (they self-pace
                    # on the per-2-ei xT chunk arrivals)
                    for _ in range(10):
                        warm()
                    # interleave: xT chunk 0, M chunk 0 (so the first
                    # projection matmuls start ~3us after DMA start),
                    # then the rest of xT (group 0's accumulation consumes
                    # them in arrival order), then the remaining M chunks.
                    # Per-engine DMA queues are FIFO, so emission order is
                    # arrival order.
                    xT = xtp.tile([128, ET, PW], BF16, tag="xT")
                    nc.sync.dma_start(xT[:, 0:2, :], xt_d.ap()[0, :, 0:2, :])
                    nc.sync.dma_start(msb[:, 0], m_d.ap()[:, 0])
                    for h in range(1, 4):
                        nc.sync.dma_start(
                            xT[:, 2 * h : 2 * h + 2, :],
                            xt_d.ap()[0, :, 2 * h : 2 * h + 2, :],
                        )
                        nc.sync.dma_start(msb[:, h], m_d.ap()[:, h])
                    for eo in range(4, ET):
                        nc.sync.dma_start(msb[:, eo], m_d.ap()[:, eo])
                    nc.sync.dma_start(
                        masks[:, :, :],
                        mask_d.ap().rearrange("p (k w) -> p k w", k=2),
                    )
                else:
                    xT = xT_next

                # ---- q' projection for the pair -> per-eo q'T [128, 512] ----
                qts = []
                for eo in range(ET):
                    pp = ps_big.tile([128, PW], F32, tag="big")
                    for ei in range(ET):
                        nc.tensor.matmul(
                            pp[:],
                            msb[:, eo, ei, :],
                            xT[:, ei, :],
                            start=(ei == 0),
                            stop=(ei == ET - 1),
                        )
                    qt = qtp.tile([128, PW], BF16, tag="qT")
                    nc.scalar.copy(qt[:], pp[:])
                    qts.append(qt)

                if p == 0:
                    # N chunks + ckc slot in behind the pair-0 q' matmuls
                    for eq in range(0, ET, 2):
                        nc.sync.dma_start(
                            nsb[:, eq : eq + 2, :], n_d.ap()[:, eq : eq + 2, :]
                        )
                    nc.sync.dma_start(ckc[:], ckc_d.ap())

                # prefetch next pair's xT behind this pair's compute
                if p + 1 < npair:
                    xT_next = xtp.tile([128, ET, PW], BF16, tag="xT")
                    nc.sync.dma_start(xT_next[:, :, :], xt_d.ap()[p + 1])

                for wi in range(2):
                    w = 2 * p + wi
                    tok0 = w * W
                    wt0 = wi * W  # token offset inside the pair tiles

                    # ---- transposed scores sT[k, q] + softmax ----
                    # causal: the kt=1 key block only serves queries q>=128;
                    # its block is computed 128 columns wide (q = 128+col).
                    expT = etp.tile([128, 2, W], BF16, tag="expT")
                    for kt in range(2):
                        qw = W if kt == 0 else 128
                        q0 = wt0 + kt * 128  # first query column needed
                        sc = ps_qk.tile([128, W], F32, tag="qk")
                        for ei in range(ET):
                            nc.tensor.matmul(
                                sc[:, 0:qw],
                                xT[:, ei, wt0 + kt * 128 : wt0 + (kt + 1) * 128],
                                qts[ei][:, q0 : q0 + qw],
                                start=(ei == 0),
                                stop=(ei == ET - 1),
                            )
                        s_sb = sp.tile([128, W], F32, tag="s")
                        nc.vector.tensor_add(
                            s_sb[:, 0:qw], sc[:, 0:qw], masks[:, kt, 0:qw]
                        )
                        # exp(SCALE*s + SCALE*ck[k]): per-key bias via ACT
                        nc.scalar.activation(
                            expT[:, kt, kt * 128 : kt * 128 + qw],
                            s_sb[:, 0:qw],
                            AF.Exp,
                            scale=SCALE,
                            bias=ckc[:, 2 * w + kt : 2 * w + kt + 1],
                        )

                    # ---- v' projection (token-major), fills PE during softmax ----
                    v_w = [
                        vp.tile([128, E], BF16, tag="v", name=f"v{kt}")
                        for kt in range(2)
                    ]
                    for kt in range(2):
                        for eoh in range(2):
                            pv = ps_big.tile([128, 512], F32, tag="big")
                            for ei in range(ET):
                                nc.tensor.matmul(
                                    pv[:],
                                    xT[:, ei, wt0 + kt * 128 : wt0 + (kt + 1) * 128],
                                    nsb[:, ei, eoh * 512 : (eoh + 1) * 512],
                                    start=(ei == 0),
                                    stop=(ei == ET - 1),
                                )
                            nc.vector.tensor_copy(
                                v_w[kt][:, eoh * 512 : (eoh + 1) * 512], pv[:]
                            )

                    # ---- softmax row sums (over k = partitions) ----
                    # qt=0 queries only attend to kt=0 keys (causal)
                    recs = []
                    for qt in range(2):
                        kts = (0,) if qt == 0 else (0, 1)
                        sm = ps_sm.tile([128, 2], F32, tag="sum")
                        for kt in kts:
                            nc.tensor.matmul(
                                sm[:],
                                expT[:, kt, qt * 128 : (qt + 1) * 128],
                                onec[:],
                                start=(kt == kts[0]),
                                stop=(kt == kts[-1]),
                            )
                        rec = smp.tile([128, 1], F32, tag="rec")
                        nc.vector.reciprocal(rec[:], sm[:, 0:1])
                        recs.append(rec)

                    # ---- out = attn @ v' (token-major), normalize in evac ----
                    for qt in range(2):
                        kts = (0,) if qt == 0 else (0, 1)
                        o_sb = op.tile([128, E], BF16, tag="o")
                        for eoh in range(2):
                            po = ps_big.tile([128, 512], F32, tag="big")
                            for kt in kts:
                                nc.tensor.matmul(
                                    po[:],
                                    expT[:, kt, qt * 128 : (qt + 1) * 128],
                                    v_w[kt][:, eoh * 512 : (eoh + 1) * 512],
                                    start=(kt == kts[0]),
                                    stop=(kt == kts[-1]),
                                )
                            # normalize-evacuate on ACT; for the final
                            # window alternate ACT/DVE per half so both
                            # engines drain the tail concurrently
                            if w < nw - 1 or eoh == 0:
                                nc.scalar.activation(
                                    o_sb[:, eoh * 512 : (eoh + 1) * 512],
                                    po[:],
                                    AF.Copy,
                                    scale=recs[qt][:],
                                )
                            else:
                                nc.vector.tensor_scalar_mul(
                                    o_sb[:, eoh * 512 : (eoh + 1) * 512],
                                    po[:],
                                    recs[qt][:],
                                )
                        # one store per qt (each DMA instruction costs
                        # ~600ns of sync-engine issue, so fewer + larger
                        # wins even on the final window's drain).  Stores
                        # are emitted after the next pair's prefetch in
                        # sync-queue order, so they never head-of-line-
                        # block it.
                        nc.sync.dma_start(
                            o_d.ap()[tok0 + qt * 128 : tok0 + (qt + 1) * 128, :],
                            o_sb[:],
                        )

    nc.compile()
    return nc


_NC_CACHE = {}


def _get_nc(nw=NW):
    if nw not in _NC_CACHE:
        _NC_CACHE[nw] = build_nc(nw)
    return _NC_CACHE[nw]


def prepare(x, Wq, bq, Wk, bk, Wv, bv, Wo, bo):
    """Host-side precompute: per-core input maps + host residual terms."""
    x = np.asarray(x, dtype=np.float32)
    B, S, _ = x.shape
    x_flat = np.ascontiguousarray(x.reshape(B * S, E))
    t_core = B * S // N_CORES
    assert t_core == T
    npair = T // PW

    f64 = np.float64
    Wq64, Wk64 = np.asarray(Wq, f64), np.asarray(Wk, f64)
    Wv64, Wo64 = np.asarray(Wv, f64), np.asarray(Wo, f64)
    bq64 = np.asarray(bq, f64)
    bv64, bo64 = np.asarray(bv, f64), np.asarray(bo, f64)

    M = (Wq64.T @ Wk64).astype(np.float32)
    N = (Wv64.T @ Wo64.T).astype(np.float32)
    # tile layouts: m[p][eo][ei][c] = M[ei*128+p, eo*128+c]; n[p][ei][c]
    m_t = np.ascontiguousarray(
        M.reshape(ET, 128, ET, 128).transpose(1, 2, 0, 3).astype(NP_BF16)
    )
    n_t = np.ascontiguousarray(
        N.reshape(ET, 128, E).transpose(1, 0, 2).astype(NP_BF16)
    )
    ck = (x_flat.astype(f64) @ (Wk64.T @ bq64)) * SCALE  # [T_total]
    orow = (bv64 @ Wo64.T + bo64).astype(np.float32)  # [E]

    common = {"m": m_t, "n": n_t}
    in_maps = []
    for i in range(N_CORES):
        xc = x_flat[i * t_core : (i + 1) * t_core]
        # xt[pair][p][ei][t] = x[pair*PW + t, ei*128 + p]
        xt = np.ascontiguousarray(
            xc.reshape(npair, PW, ET, 128).transpose(0, 3, 2, 1).astype(NP_BF16)
        )
        in_maps.append(
            {
                "xt": xt,
                # ck columns: [128, 2*nw], one column per 128-token chunk
                "ckc": np.ascontiguousarray(
                    ck[i * t_core : (i + 1) * t_core]
                    .astype(np.float32)
                    .reshape(2 * NW, 128)
                    .T
                ),
                **common,
            }
        )
    return in_maps, orow, x_flat, (B, S)


def kernel(x, Wq, bq, Wk, bk, Wv, bv, Wo, bo):
    in_maps, orow, x_flat, (B, S) = prepare(x, Wq, bq, Wk, bk, Wv, bv, Wo, bo)
    nc = _get_nc()
    res = run_bass_kernel_spmd(nc, in_maps, core_ids=list(range(N_CORES)))
    out = np.concatenate(
        [np.asarray(res.results[i]["o"]).astype(np.float32) for i in range(N_CORES)],
        axis=0,
    )
    out += orow[None, :]
    out += x_flat
    return out.reshape(B, S, E).astype(np.float32)


# revision 21
# speedup vs baseline: 1.0494x; 1.0024x over previous
"""Trainium2 Bass kernel for windowed (local) causal self-attention.

Reference computation (per batch element, fp32):
    q = x @ Wq.T + bq ; k = x @ Wk.T + bk ; v = x @ Wv.T + bv
    per non-overlapping window of 256 tokens:
        attn = softmax(causal_mask(q k^T * HEAD_DIM**-0.5))
        out  = attn @ v
    o = out @ Wo.T + bo + x

Algebraic restructure (no head split in this module, softmax rows sum to 1):
    scores = q k^T = x M x^T + cq 1^T + 1 ck^T + bq.bk,  M  = Wq^T Wk
        cq = x (Wq^T bk)  [per-QUERY shift: cancels in softmax, dropped]
        ck = x (Wk^T bq)  [per-KEY: folded into the ACT exp bias]
    o = attn (x N) + (bv Wo^T + bo) + x,      N  = Wv^T Wo^T
so only TWO E x E projections remain on device (q' = x M and v' = x N);
M, N, ck and the constant output row are computed on the host in float64.
The residual + constant row are also added on the host.

Sharding: data-parallel over (batch, window): 64 window-blocks of 256
tokens -> 8 cores x 8 windows.  M, N replicated.

Per-core kernel strategy:
  - ALL matmul operands are bf16 (PE streams 1 col/cycle regardless of
    dtype, so bf16 matches fp32r rate at half the DMA/SBUF traffic and,
    critically, every LDWEIGHTS is a cheap 97ns FWL load -- no fp32-mode
    two-pass weight loads anywhere).  PSUM accumulation is fp32; the
    softmax chain (mask add, exp, row sums, reciprocal) stays fp32
    except the stored exp values (bf16, 0.4% rel err on attn weights).
  - scores are computed TRANSPOSED, sT[k, q] = x_k . q'_q, so no PE
    transposes of the attention matrix are needed: exp(sT) chunks serve
    directly as the stationary operand of out = attn @ v'.
  - causal block-sparsity: the kt=1 key block only serves queries q>=128,
    so its score matmuls/exp narrow to 128 columns and the (qt=0, kt=1)
    output/sum matmuls are skipped.
  - softmax row sums become N=2 matmuls (expT^T @ ones2) accumulated over
    k-chunks; normalization is folded into the ACT output evacuation as a
    per-partition scale (1/sum).
  - q'-projection is window-PAIRED (moving 512 tokens) to halve its
    instruction count and PE weight-load switches.  qT is split into
    per-eo tiles so the first score matmuls of a pair start as soon as
    their own chunk is evacuated (no whole-tile dependency stall).
  - v' is computed token-major between the score matmuls and the
    attention matmuls so the PE stays busy through the softmax chain.
  - ALL dram tensors are pre-arranged on the host into the exact SBUF
    tile layouts, so every DMA moves contiguous per-partition lines
    (1-16KB packets instead of 256B gather packets).  Boot: tiny consts,
    then pair-0 xT (2 contiguous halves), then M in 8 eo-chunks paced
    with tiny PE warmup matmuls; the first projection matmuls start as
    soon as (xT, M chunk 0) land.  N chunks and the next pair's xT are
    prefetched behind pair-0 compute; output stores (bf16, host upcasts
    and adds the residual) are deferred behind the next pair's loads.
"""
import sys

sys.path.insert(0, "/opt/trn_rl_repo")

import numpy as np
import ml_dtypes

import concourse.bass as bass
import concourse.bacc as bacc
import concourse.mybir as mybir
import concourse.tile as tile
from concourse.bass_utils import run_bass_kernel_spmd

F32 = mybir.dt.float32
BF16 = mybir.dt.bfloat16
NP_BF16 = ml_dtypes.bfloat16
AF = mybir.ActivationFunctionType

E = 1024          # embed dim
ET = E // 128     # e-tiles
W = 256           # window size
NW = 8            # windows per core
T = NW * W        # tokens per core
N_CORES = 8
SCALE = (E // 16) ** (-0.5)  # HEAD_DIM ** -0.5 = 0.125
NEG = -1.0e30
PW = 2 * W        # tokens per window pair


def build_nc(nw=NW):
    t_core = nw * W
    npair = nw // 2
    nc = bacc.Bacc("TRN2", target_bir_lowering=False, debug=False)

    # host-prearranged layouts (exactly the SBUF tile layouts):
    #   xt[pair][p][ei][t]  = x[pair*PW + t, ei*128 + p]
    #   m[p][eo][ei][c]     = M[ei*128 + p, eo*128 + c]
    #   n[p][ei][c]         = N[ei*128 + p, c]
    xt_d = nc.dram_tensor("xt", [npair, 128, ET, PW], BF16, kind="ExternalInput")
    m_d = nc.dram_tensor("m", [128, ET, ET, 128], BF16, kind="ExternalInput")
    n_d = nc.dram_tensor("n", [128, ET, E], BF16, kind="ExternalInput")
    # ck * SCALE laid out as one [128] column per 128-token chunk
    ckc_d = nc.dram_tensor("ckc", [128, 2 * nw], F32, kind="ExternalInput")
    o_d = nc.dram_tensor("o", [t_core, E], BF16, kind="ExternalOutput")

    # transposed causal mask constants applied to sT[k, q] = score(q, k).
    # kt=0: full [k, 0:256] triangle.  kt=1: only queries q>=128 are kept
    # downstream, stored in columns 0:128 (q = 128 + col).  (The bq.bk
    # score constant and the per-query cq row are dropped: uniform per-row
    # logit shifts cancel in softmax.)
    mask_np = np.full((2, 128, W), NEG, dtype=np.float32)
    k_idx = np.arange(128)[:, None]
    mask_np[0][k_idx <= np.arange(W)[None, :]] = 0.0
    mask_np[1][:, 0:128][k_idx <= np.arange(128)[None, :]] = 0.0
    # single [128, 512] row-contiguous image (2KB DMA lines)
    mask_d = nc.inline_tensor(
        np.ascontiguousarray(mask_np.transpose(1, 0, 2).reshape(128, 2 * W)), "mask"
    )

    with tile.TileContext(nc) as tc:
        with (
            tc.tile_pool(name="wp", bufs=1) as wp,
            tc.tile_pool(name="cp", bufs=1) as cp,
            tc.tile_pool(name="xtp", bufs=2) as xtp,
            tc.tile_pool(name="qtp", bufs=2 * ET) as qtp,
            tc.tile_pool(name="etp", bufs=2) as etp,
            tc.tile_pool(name="sp", bufs=4) as sp,
            tc.tile_pool(name="vp", bufs=4) as vp,
            tc.tile_pool(name="smp", bufs=8) as smp,
            tc.tile_pool(name="op", bufs=4) as op,
            tc.tile_pool(name="ps_qk", bufs=2, space=bass.MemorySpace.PSUM) as ps_qk,
            tc.tile_pool(name="ps_big", bufs=5, space=bass.MemorySpace.PSUM) as ps_big,
            tc.tile_pool(name="ps_sm", bufs=1, space=bass.MemorySpace.PSUM) as ps_sm,
        ):
            # ---- resident constants ----
            masks = cp.tile([128, 2, W], F32, tag="mask")
            onec = cp.tile([128, 2], BF16, tag="onec")
            ckc = cp.tile([128, 2 * nw], F32, tag="ckc")
            # warm tile: DVE-memset (no DMA dependency) so the PE warmup
            # stream can start at engine-ready, flipping the HAM clock gate
            # to 8/8 before the first real matmul issues
            wsb = cp.tile([128, 512], BF16, tag="wsb")

            # ---- resident weights in tile layout ----
            msb = wp.tile([128, ET, ET, 128], BF16, tag="m", name="msb")
            nsb = wp.tile([128, ET, E], BF16, tag="n", name="nsb")

            def warm():
                # a PE-busy matmul: 256 bf16 moving columns keeps the PE
                # activity monitor's busy-duty high through boot.  Uses
                # the (boot-idle) score bank so ps_big gets 5 buffers.
                wps = ps_big.tile([128, 512], F32, tag="big", name="warm")
                nc.tensor.matmul(
                    wps[:], wsb[:, 0:128], wsb[:], start=True, stop=True
                )

            xT_next = None
            for p in range(npair):
                ptok0 = p * PW

                # ---- xT[p, ei, t] for the pair (host-prearranged) ----
                if p == 0:
                    nc.vector.memset(wsb[:], 1.0)
                    nc.vector.memset(onec[:], 1.0)
                    # upfront warmup matmuls start warming the HAM clock
                    # gate at engine-ready and bridge the DMA wait until
                    # the q' eo-group 0 matmuls take over (they self-pace
                    # on the per-2-ei xT chunk arrivals)
                    for _ in range(10):
                        warm()
                    # interleave: xT chunk 0, M chunk 0 (so the first
                    # projection matmuls start ~3us after DMA start),
                    # then the rest of xT (group 0's accumulation consumes
                    # them in arrival order), then the remaining M chunks.
                    # Per-engine DMA queues are FIFO, so emission order is
                    # arrival order.
                    xT = xtp.tile([128, ET, PW], BF16, tag="xT")
                    nc.sync.dma_start(xT[:, 0:2, :], xt_d.ap()[0, :, 0:2, :])
                    nc.sync.dma_start(msb[:, 0], m_d.ap()[:, 0])
                    for h in range(1, 4):
                        nc.sync.dma_start(
                            xT[:, 2 * h : 2 * h + 2, :],
                            xt_d.ap()[0, :, 2 * h : 2 * h + 2, :],
                        )
                        nc.sync.dma_start(msb[:, h], m_d.ap()[:, h])
                    for eo in range(4, ET):
                        nc.sync.dma_start(msb[:, eo], m_d.ap()[:, eo])
                    nc.sync.dma_start(
                        masks[:, :, :],
                        mask_d.ap().rearrange("p (k w) -> p k w", k=2),
                    )
                else:
                    xT = xT_next

                # ---- q' projection for the pair -> per-eo q'T [128, 512] ----
                qts = []
                for eo in range(ET):
                    pp = ps_big.tile([128, PW], F32, tag="big")
                    for ei in range(ET):
                        nc.tensor.matmul(
                            pp[:],
                            msb[:, eo, ei, :],
                            xT[:, ei, :],
                            start=(ei == 0),
                            stop=(ei == ET - 1),
                        )
                    qt = qtp.tile([128, PW], BF16, tag="qT")
                    nc.scalar.copy(qt[:], pp[:])
                    qts.append(qt)

                if p == 0:
                    # N chunks + ckc slot in behind the pair-0 q' matmuls
                    for eq in range(0, ET, 2):
                        nc.sync.dma_start(
                            nsb[:, eq : eq + 2, :], n_d.ap()[:, eq : eq + 2, :]
                        )
                    nc.sync.dma_start(ckc[:], ckc_d.ap())

                # prefetch next pair's xT behind this pair's compute
                if p + 1 < npair:
                    xT_next = xtp.tile([128, ET, PW], BF16, tag="xT")
                    nc.sync.dma_start(xT_next[:, :, :], xt_d.ap()[p + 1])

                for wi in range(2):
                    w = 2 * p + wi
                    tok0 = w * W
                    wt0 = wi * W  # token offset inside the pair tiles

                    # ---- transposed scores sT[k, q] + softmax ----
                    # causal: the kt=1 key block only serves queries q>=128;
                    # its block is computed 128 columns wide (q = 128+col).
                    expT = etp.tile([128, 2, W], BF16, tag="expT")
                    for kt in range(2):
                        qw = W if kt == 0 else 128
                        q0 = wt0 + kt * 128  # first query column needed
                        sc = ps_qk.tile([128, W], F32, tag="qk")
                        for ei in range(ET):
                            nc.tensor.matmul(
                                sc[:, 0:qw],
                                xT[:, ei, wt0 + kt * 128 : wt0 + (kt + 1) * 128],
                                qts[ei][:, q0 : q0 + qw],
                                start=(ei == 0),
                                stop=(ei == ET - 1),
                            )
                        s_sb = sp.tile([128, W], F32, tag="s")
                        nc.vector.tensor_add(
                            s_sb[:, 0:qw], sc[:, 0:qw], masks[:, kt, 0:qw]
                        )
                        # exp(SCALE*s + SCALE*ck[k]): per-key bias via ACT
                        nc.scalar.activation(
                            expT[:, kt, kt * 128 : kt * 128 + qw],
                            s_sb[:, 0:qw],
                            AF.Exp,
                            scale=SCALE,
                            bias=ckc[:, 2 * w + kt : 2 * w + kt + 1],
                        )

                    # ---- v' projection (token-major), fills PE during softmax ----
                    v_w = [
                        vp.tile([128, E], BF16, tag="v", name=f"v{kt}")
                        for kt in range(2)
                    ]
                    for kt in range(2):
                        for eoh in range(2):
                            pv = ps_big.tile([128, 512], F32, tag="big")
                            for ei in range(ET):
                                nc.tensor.matmul(
                                    pv[:],
                                    xT[:, ei, wt0 + kt * 128 : wt0 + (kt + 1) * 128],
                                    nsb[:, ei, eoh * 512 : (eoh + 1) * 512],
                                    start=(ei == 0),
                                    stop=(ei == ET - 1),
                                )
                            nc.vector.tensor_copy(
                                v_w[kt][:, eoh * 512 : (eoh + 1) * 512], pv[:]
                            )

                    # ---- softmax row sums (over k = partitions) ----
                    # qt=0 queries only attend to kt=0 keys (causal).
                    # Final window: qt1 (the longer chain) goes first so
                    # its evac + store drain while qt0 still computes.
                    qt_order = (0, 1) if w < nw - 1 else (1, 0)
                    recs = [None, None]
                    for qt in qt_order:
                        kts = (0,) if qt == 0 else (0, 1)
                        sm = ps_sm.tile([128, 2], F32, tag="sum")
                        for kt in kts:
                            nc.tensor.matmul(
                                sm[:],
                                expT[:, kt, qt * 128 : (qt + 1) * 128],
                                onec[:],
                                start=(kt == kts[0]),
                                stop=(kt == kts[-1]),
                            )
                        rec = smp.tile([128, 1], F32, tag="rec")
                        nc.vector.reciprocal(rec[:], sm[:, 0:1])
                        recs[qt] = rec

                    # ---- out = attn @ v' (token-major), normalize in evac ----
                    for qt in qt_order:
                        kts = (0,) if qt == 0 else (0, 1)
                        o_sb = op.tile([128, E], BF16, tag="o")
                        for eoh in range(2):
                            po = ps_big.tile([128, 512], F32, tag="big")
                            for kt in kts:
                                nc.tensor.matmul(
                                    po[:],
                                    expT[:, kt, qt * 128 : (qt + 1) * 128],
                                    v_w[kt][:, eoh * 512 : (eoh + 1) * 512],
                                    start=(kt == kts[0]),
                                    stop=(kt == kts[-1]),
                                )
                            # normalize-evacuate on ACT; for the final
                            # window alternate ACT/DVE per half so both
                            # engines drain the tail concurrently
                            if w < nw - 1 or eoh == 0:
                                nc.scalar.activation(
                                    o_sb[:, eoh * 512 : (eoh + 1) * 512],
                                    po[:],
                                    AF.Copy,
                                    scale=recs[qt][:],
                                )
                            else:
                                nc.vector.tensor_scalar_mul(
                                    o_sb[:, eoh * 512 : (eoh + 1) * 512],
                                    po[:],
                                    recs[qt][:],
                                )
                        # one store per qt (each DMA instruction costs
                        # ~600ns of sync-engine issue, so fewer + larger
                        # wins even on the final window's drain).  Stores
                        # are emitted after the next pair's prefetch in
                        # sync-queue order, so they never head-of-line-
                        # block it.
                        nc.sync.dma_start(
                            o_d.ap()[tok0 + qt * 128 : tok0 + (qt + 1) * 128, :],
                            o_sb[:],
                        )

    nc.compile()
    return nc


_NC_CACHE = {}


def _get_nc(nw=NW):
    if nw not in _NC_CACHE:
        _NC_CACHE[nw] = build_nc(nw)
    return _NC_CACHE[nw]


def prepare(x, Wq, bq, Wk, bk, Wv, bv, Wo, bo):
    """Host-side precompute: per-core input maps + host residual terms."""
    x = np.asarray(x, dtype=np.float32)
    B, S, _ = x.shape
    x_flat = np.ascontiguousarray(x.reshape(B * S, E))
    t_core = B * S // N_CORES
    assert t_core == T
    npair = T // PW

    f64 = np.float64
    Wq64, Wk64 = np.asarray(Wq, f64), np.asarray(Wk, f64)
    Wv64, Wo64 = np.asarray(Wv, f64), np.asarray(Wo, f64)
    bq64 = np.asarray(bq, f64)
    bv64, bo64 = np.asarray(bv, f64), np.asarray(bo, f64)

    M = (Wq64.T @ Wk64).astype(np.float32)
    N = (Wv64.T @ Wo64.T).astype(np.float32)
    # tile layouts: m[p][eo][ei][c] = M[ei*128+p, eo*128+c]; n[p][ei][c]
    m_t = np.ascontiguousarray(
        M.reshape(ET, 128, ET, 128).transpose(1, 2, 0, 3).astype(NP_BF16)
    )
    n_t = np.ascontiguousarray(
        N.reshape(ET, 128, E).transpose(1, 0, 2).astype(NP_BF16)
    )
    ck = (x_flat.astype(f64) @ (Wk64.T @ bq64)) * SCALE  # [T_total]
    orow = (bv64 @ Wo64.T + bo64).astype(np.float32)  # [E]

    common = {"m": m_t, "n": n_t}
    in_maps = []
    for i in range(N_CORES):
        xc = x_flat[i * t_core : (i + 1) * t_core]
        # xt[pair][p][ei][t] = x[pair*PW + t, ei*128 + p]
        xt = np.ascontiguousarray(
            xc.reshape(npair, PW, ET, 128).transpose(0, 3, 2, 1).astype(NP_BF16)
        )
        in_maps.append(
            {
                "xt": xt,
                # ck columns: [128, 2*nw], one column per 128-token chunk
                "ckc": np.ascontiguousarray(
                    ck[i * t_core : (i + 1) * t_core]
                    .astype(np.float32)
                    .reshape(2 * NW, 128)
                    .T
                ),
                **common,
            }
        )
    return in_maps, orow, x_flat, (B, S)


def kernel(x, Wq, bq, Wk, bk, Wv, bv, Wo, bo):
    in_maps, orow, x_flat, (B, S) = prepare(x, Wq, bq, Wk, bk, Wv, bv, Wo, bo)
    nc = _get_nc()
    res = run_bass_kernel_spmd(nc, in_maps, core_ids=list(range(N_CORES)))
    out = np.concatenate(
        [np.asarray(res.results[i]["o"]).astype(np.float32) for i in range(N_CORES)],
        axis=0,
    )
    out += orow[None, :]
    out += x_flat
    return out.reshape(B, S, E).astype(np.float32)


# revision 22
# speedup vs baseline: 1.0516x; 1.0021x over previous
"""Trainium2 Bass kernel for windowed (local) causal self-attention.

Reference computation (per batch element, fp32):
    q = x @ Wq.T + bq ; k = x @ Wk.T + bk ; v = x @ Wv.T + bv
    per non-overlapping window of 256 tokens:
        attn = softmax(causal_mask(q k^T * HEAD_DIM**-0.5))
        out  = attn @ v
    o = out @ Wo.T + bo + x

Algebraic restructure (no head split in this module, softmax rows sum to 1):
    scores = q k^T = x M x^T + cq 1^T + 1 ck^T + bq.bk,  M  = Wq^T Wk
        cq = x (Wq^T bk)  [per-QUERY shift: cancels in softmax, dropped]
        ck = x (Wk^T bq)  [per-KEY: folded into the ACT exp bias]
    o = attn (x N) + (bv Wo^T + bo) + x,      N  = Wv^T Wo^T
so only TWO E x E projections remain on device (q' = x M and v' = x N);
M, N, ck and the constant output row are computed on the host in float64.
The residual + constant row are also added on the host.

Sharding: data-parallel over (batch, window): 64 window-blocks of 256
tokens -> 8 cores x 8 windows.  M, N replicated.

Per-core kernel strategy:
  - ALL matmul operands are bf16 (PE streams 1 col/cycle regardless of
    dtype, so bf16 matches fp32r rate at half the DMA/SBUF traffic and,
    critically, every LDWEIGHTS is a cheap 97ns FWL load -- no fp32-mode
    two-pass weight loads anywhere).  PSUM accumulation is fp32; the
    softmax chain (mask add, exp, row sums, reciprocal) stays fp32
    except the stored exp values (bf16, 0.4% rel err on attn weights).
  - scores are computed TRANSPOSED, sT[k, q] = x_k . q'_q, so no PE
    transposes of the attention matrix are needed: exp(sT) chunks serve
    directly as the stationary operand of out = attn @ v'.
  - causal block-sparsity: the kt=1 key block only serves queries q>=128,
    so its score matmuls/exp narrow to 128 columns and the (qt=0, kt=1)
    output/sum matmuls are skipped.
  - softmax row sums become N=2 matmuls (expT^T @ ones2) accumulated over
    k-chunks; normalization is folded into the ACT output evacuation as a
    per-partition scale (1/sum).
  - q'-projection is window-PAIRED (moving 512 tokens) to halve its
    instruction count and PE weight-load switches.  qT is split into
    per-eo tiles so the first score matmuls of a pair start as soon as
    their own chunk is evacuated (no whole-tile dependency stall).
  - v' is computed token-major between the score matmuls and the
    attention matmuls so the PE stays busy through the softmax chain.
  - ALL dram tensors are pre-arranged on the host into the exact SBUF
    tile layouts, so every DMA moves contiguous per-partition lines
    (1-16KB packets instead of 256B gather packets).  Boot: tiny consts,
    then pair-0 xT (2 contiguous halves), then M in 8 eo-chunks paced
    with tiny PE warmup matmuls; the first projection matmuls start as
    soon as (xT, M chunk 0) land.  N chunks and the next pair's xT are
    prefetched behind pair-0 compute; output stores (bf16, host upcasts
    and adds the residual) are deferred behind the next pair's loads.
"""
import sys

sys.path.insert(0, "/opt/trn_rl_repo")

import numpy as np
import ml_dtypes

import concourse.bass as bass
import concourse.bacc as bacc
import concourse.mybir as mybir
import concourse.tile as tile
from concourse.bass_utils import run_bass_kernel_spmd

F32 = mybir.dt.float32
BF16 = mybir.dt.bfloat16
NP_BF16 = ml_dtypes.bfloat16
AF = mybir.ActivationFunctionType

E = 1024          # embed dim
ET = E // 128     # e-tiles
W = 256           # window size
NW = 8            # windows per core
T = NW * W        # tokens per core
N_CORES = 8
SCALE = (E // 16) ** (-0.5)  # HEAD_DIM ** -0.5 = 0.125
NEG = -1.0e30
PW = 2 * W        # tokens per window pair


def build_nc(nw=NW):
    t_core = nw * W
    npair = nw // 2
    nc = bacc.Bacc("TRN2", target_bir_lowering=False, debug=False)

    # host-prearranged layouts (exactly the SBUF tile layouts):
    #   xt[pair][p][ei][t]  = x[pair*PW + t, ei*128 + p]
    #   m[p][eo][ei][c]     = M[ei*128 + p, eo*128 + c]
    #   n[p][ei][c]         = N[ei*128 + p, c]
    xt_d = nc.dram_tensor("xt", [npair, 128, ET, PW], BF16, kind="ExternalInput")
    m_d = nc.dram_tensor("m", [128, ET, ET, 128], BF16, kind="ExternalInput")
    n_d = nc.dram_tensor("n", [128, ET, E], BF16, kind="ExternalInput")
    # ck * SCALE laid out as one [128] column per 128-token chunk
    ckc_d = nc.dram_tensor("ckc", [128, 2 * nw], F32, kind="ExternalInput")
    o_d = nc.dram_tensor("o", [t_core, E], BF16, kind="ExternalOutput")

    # transposed causal mask constants applied to sT[k, q] = score(q, k).
    # kt=0: full [k, 0:256] triangle.  kt=1: only queries q>=128 are kept
    # downstream, stored in columns 0:128 (q = 128 + col).  (The bq.bk
    # score constant and the per-query cq row are dropped: uniform per-row
    # logit shifts cancel in softmax.)
    mask_np = np.full((2, 128, W), NEG, dtype=np.float32)
    k_idx = np.arange(128)[:, None]
    mask_np[0][k_idx <= np.arange(W)[None, :]] = 0.0
    mask_np[1][:, 0:128][k_idx <= np.arange(128)[None, :]] = 0.0
    # single [128, 512] row-contiguous image (2KB DMA lines)
    mask_d = nc.inline_tensor(
        np.ascontiguousarray(mask_np.transpose(1, 0, 2).reshape(128, 2 * W)), "mask"
    )

    with tile.TileContext(nc) as tc:
        with (
            tc.tile_pool(name="wp", bufs=1) as wp,
            tc.tile_pool(name="cp", bufs=1) as cp,
            tc.tile_pool(name="xtp", bufs=2) as xtp,
            tc.tile_pool(name="qtp", bufs=2 * ET) as qtp,
            tc.tile_pool(name="etp", bufs=2) as etp,
            tc.tile_pool(name="sp", bufs=4) as sp,
            tc.tile_pool(name="vp", bufs=4) as vp,
            tc.tile_pool(name="smp", bufs=8) as smp,
            tc.tile_pool(name="op", bufs=4) as op,
            tc.tile_pool(name="ps_qk", bufs=2, space=bass.MemorySpace.PSUM) as ps_qk,
            tc.tile_pool(name="ps_big", bufs=5, space=bass.MemorySpace.PSUM) as ps_big,
            tc.tile_pool(name="ps_sm", bufs=1, space=bass.MemorySpace.PSUM) as ps_sm,
        ):
            # ---- resident constants ----
            masks = cp.tile([128, 2, W], F32, tag="mask")
            onec = cp.tile([128, 2], BF16, tag="onec")
            ckc = cp.tile([128, 2 * nw], F32, tag="ckc")
            # warm tile: DVE-memset (no DMA dependency) so the PE warmup
            # stream can start at engine-ready, flipping the HAM clock gate
            # to 8/8 before the first real matmul issues
            wsb = cp.tile([128, 512], BF16, tag="wsb")

            # ---- resident weights in tile layout ----
            msb = wp.tile([128, ET, ET, 128], BF16, tag="m", name="msb")
            nsb = wp.tile([128, ET, E], BF16, tag="n", name="nsb")

            def warm():
                # a PE-busy matmul: 256 bf16 moving columns keeps the PE
                # activity monitor's busy-duty high through boot.  Uses
                # the (boot-idle) score bank so ps_big gets 5 buffers.
                wps = ps_big.tile([128, 512], F32, tag="big", name="warm")
                nc.tensor.matmul(
                    wps[:], wsb[:, 0:128], wsb[:], start=True, stop=True
                )

            xT_next = None
            for p in range(npair):
                ptok0 = p * PW

                # ---- xT[p, ei, t] for the pair (host-prearranged) ----
                if p == 0:
                    nc.vector.memset(wsb[:], 1.0)
                    nc.vector.memset(onec[:], 1.0)
                    # upfront warmup matmuls start warming the HAM clock
                    # gate at engine-ready and bridge the DMA wait until
                    # the q' eo-group 0 matmuls take over (they self-pace
                    # on the per-2-ei xT chunk arrivals)
                    for _ in range(10):
                        warm()
                    # interleave: xT chunk 0, M chunk 0 (so the first
                    # projection matmuls start ~3us after DMA start),
                    # then the rest of xT (group 0's accumulation consumes
                    # them in arrival order), then the remaining M chunks.
                    # Per-engine DMA queues are FIFO, so emission order is
                    # arrival order.
                    xT = xtp.tile([128, ET, PW], BF16, tag="xT")
                    nc.sync.dma_start(xT[:, 0:2, :], xt_d.ap()[0, :, 0:2, :])
                    nc.sync.dma_start(msb[:, 0], m_d.ap()[:, 0])
                    nc.sync.dma_start(msb[:, 1], m_d.ap()[:, 1])
                    for h in range(1, 4):
                        nc.sync.dma_start(
                            xT[:, 2 * h : 2 * h + 2, :],
                            xt_d.ap()[0, :, 2 * h : 2 * h + 2, :],
                        )
                        nc.sync.dma_start(msb[:, h + 1], m_d.ap()[:, h + 1])
                    for eo in range(5, ET):
                        nc.sync.dma_start(msb[:, eo], m_d.ap()[:, eo])
                    nc.sync.dma_start(
                        masks[:, :, :],
                        mask_d.ap().rearrange("p (k w) -> p k w", k=2),
                    )
                else:
                    xT = xT_next

                # ---- q' projection for the pair -> per-eo q'T [128, 512] ----
                qts = []
                for eo in range(ET):
                    pp = ps_big.tile([128, PW], F32, tag="big")
                    for ei in range(ET):
                        nc.tensor.matmul(
                            pp[:],
                            msb[:, eo, ei, :],
                            xT[:, ei, :],
                            start=(ei == 0),
                            stop=(ei == ET - 1),
                        )
                    qt = qtp.tile([128, PW], BF16, tag="qT")
                    nc.scalar.copy(qt[:], pp[:])
                    qts.append(qt)

                if p == 0:
                    # N chunks + ckc slot in behind the pair-0 q' matmuls
                    for eq in range(0, ET, 2):
                        nc.sync.dma_start(
                            nsb[:, eq : eq + 2, :], n_d.ap()[:, eq : eq + 2, :]
                        )
                    nc.sync.dma_start(ckc[:], ckc_d.ap())

                # prefetch next pair's xT behind this pair's compute
                if p + 1 < npair:
                    xT_next = xtp.tile([128, ET, PW], BF16, tag="xT")
                    nc.sync.dma_start(xT_next[:, :, :], xt_d.ap()[p + 1])

                for wi in range(2):
                    w = 2 * p + wi
                    tok0 = w * W
                    wt0 = wi * W  # token offset inside the pair tiles

                    # ---- transposed scores sT[k, q] + softmax ----
                    # causal: the kt=1 key block only serves queries q>=128;
                    # its block is computed 128 columns wide (q = 128+col).
                    expT = etp.tile([128, 2, W], BF16, tag="expT")
                    for kt in range(2):
                        qw = W if kt == 0 else 128
                        q0 = wt0 + kt * 128  # first query column needed
                        sc = ps_qk.tile([128, W], F32, tag="qk")
                        for ei in range(ET):
                            nc.tensor.matmul(
                                sc[:, 0:qw],
                                xT[:, ei, wt0 + kt * 128 : wt0 + (kt + 1) * 128],
                                qts[ei][:, q0 : q0 + qw],
                                start=(ei == 0),
                                stop=(ei == ET - 1),
                            )
                        s_sb = sp.tile([128, W], F32, tag="s")
                        nc.vector.tensor_add(
                            s_sb[:, 0:qw], sc[:, 0:qw], masks[:, kt, 0:qw]
                        )
                        # exp(SCALE*s + SCALE*ck[k]): per-key bias via ACT
                        nc.scalar.activation(
                            expT[:, kt, kt * 128 : kt * 128 + qw],
                            s_sb[:, 0:qw],
                            AF.Exp,
                            scale=SCALE,
                            bias=ckc[:, 2 * w + kt : 2 * w + kt + 1],
                        )

                    # ---- v' projection (token-major), fills PE during softmax ----
                    v_w = [
                        vp.tile([128, E], BF16, tag="v", name=f"v{kt}")
                        for kt in range(2)
                    ]
                    for kt in range(2):
                        for eoh in range(2):
                            pv = ps_big.tile([128, 512], F32, tag="big")
                            for ei in range(ET):
                                nc.tensor.matmul(
                                    pv[:],
                                    xT[:, ei, wt0 + kt * 128 : wt0 + (kt + 1) * 128],
                                    nsb[:, ei, eoh * 512 : (eoh + 1) * 512],
                                    start=(ei == 0),
                                    stop=(ei == ET - 1),
                                )
                            nc.vector.tensor_copy(
                                v_w[kt][:, eoh * 512 : (eoh + 1) * 512], pv[:]
                            )

                    # ---- softmax row sums (over k = partitions) ----
                    # qt=0 queries only attend to kt=0 keys (causal).
                    # Final window: qt1 (the longer chain) goes first so
                    # its evac + store drain while qt0 still computes.
                    qt_order = (0, 1) if w < nw - 1 else (1, 0)
                    recs = [None, None]
                    for qt in qt_order:
                        kts = (0,) if qt == 0 else (0, 1)
                        sm = ps_sm.tile([128, 2], F32, tag="sum")
                        for kt in kts:
                            nc.tensor.matmul(
                                sm[:],
                                expT[:, kt, qt * 128 : (qt + 1) * 128],
                                onec[:],
                                start=(kt == kts[0]),
                                stop=(kt == kts[-1]),
                            )
                        rec = smp.tile([128, 1], F32, tag="rec")
                        nc.vector.reciprocal(rec[:], sm[:, 0:1])
                        recs[qt] = rec

                    # ---- out = attn @ v' (token-major), normalize in evac ----
                    for qt in qt_order:
                        kts = (0,) if qt == 0 else (0, 1)
                        o_sb = op.tile([128, E], BF16, tag="o")
                        for eoh in range(2):
                            po = ps_big.tile([128, 512], F32, tag="big")
                            for kt in kts:
                                nc.tensor.matmul(
                                    po[:],
                                    expT[:, kt, qt * 128 : (qt + 1) * 128],
                                    v_w[kt][:, eoh * 512 : (eoh + 1) * 512],
                                    start=(kt == kts[0]),
                                    stop=(kt == kts[-1]),
                                )
                            # normalize-evacuate on ACT; for the final
                            # window alternate ACT/DVE per half so both
                            # engines drain the tail concurrently
                            if w < nw - 1 or eoh == 0:
                                nc.scalar.activation(
                                    o_sb[:, eoh * 512 : (eoh + 1) * 512],
                                    po[:],
                                    AF.Copy,
                                    scale=recs[qt][:],
                                )
                            else:
                                nc.vector.tensor_scalar_mul(
                                    o_sb[:, eoh * 512 : (eoh + 1) * 512],
                                    po[:],
                                    recs[qt][:],
                                )
                        # one store per qt (each DMA instruction costs
                        # ~600ns of sync-engine issue, so fewer + larger
                        # wins even on the final window's drain).  Stores
                        # are emitted after the next pair's prefetch in
                        # sync-queue order, so they never head-of-line-
                        # block it.
                        nc.sync.dma_start(
                            o_d.ap()[tok0 + qt * 128 : tok0 + (qt + 1) * 128, :],
                            o_sb[:],
                        )

    nc.compile()
    return nc


_NC_CACHE = {}


def _get_nc(nw=NW):
    if nw not in _NC_CACHE:
        _NC_CACHE[nw] = build_nc(nw)
    return _NC_CACHE[nw]


def prepare(x, Wq, bq, Wk, bk, Wv, bv, Wo, bo):
    """Host-side precompute: per-core input maps + host residual terms."""
    x = np.asarray(x, dtype=np.float32)
    B, S, _ = x.shape
    x_flat = np.ascontiguousarray(x.reshape(B * S, E))
    t_core = B * S // N_CORES
    assert t_core == T
    npair = T // PW

    f64 = np.float64
    Wq64, Wk64 = np.asarray(Wq, f64), np.asarray(Wk, f64)
    Wv64, Wo64 = np.asarray(Wv, f64), np.asarray(Wo, f64)
    bq64 = np.asarray(bq, f64)
    bv64, bo64 = np.asarray(bv, f64), np.asarray(bo, f64)

    M = (Wq64.T @ Wk64).astype(np.float32)
    N = (Wv64.T @ Wo64.T).astype(np.float32)
    # tile layouts: m[p][eo][ei][c] = M[ei*128+p, eo*128+c]; n[p][ei][c]
    m_t = np.ascontiguousarray(
        M.reshape(ET, 128, ET, 128).transpose(1, 2, 0, 3).astype(NP_BF16)
    )
    n_t = np.ascontiguousarray(
        N.reshape(ET, 128, E).transpose(1, 0, 2).astype(NP_BF16)
    )
    ck = (x_flat.astype(f64) @ (Wk64.T @ bq64)) * SCALE  # [T_total]
    orow = (bv64 @ Wo64.T + bo64).astype(np.float32)  # [E]

    common = {"m": m_t, "n": n_t}
    in_maps = []
    for i in range(N_CORES):
        xc = x_flat[i * t_core : (i + 1) * t_core]
        # xt[pair][p][ei][t] = x[pair*PW + t, ei*128 + p]
        xt = np.ascontiguousarray(
            xc.reshape(npair, PW, ET, 128).transpose(0, 3, 2, 1).astype(NP_BF16)
        )
        in_maps.append(
            {
                "xt": xt,
                # ck columns: [128, 2*nw], one column per 128-token chunk
                "ckc": np.ascontiguousarray(
                    ck[i * t_core : (i + 1) * t_core]
                    .astype(np.float32)
                    .reshape(2 * NW, 128)
                    .T
                ),
                **common,
            }
        )
    return in_maps, orow, x_flat, (B, S)


def kernel(x, Wq, bq, Wk, bk, Wv, bv, Wo, bo):
    in_maps, orow, x_flat, (B, S) = prepare(x, Wq, bq, Wk, bk, Wv, bv, Wo, bo)
    nc = _get_nc()
    res = run_bass_kernel_spmd(nc, in_maps, core_ids=list(range(N_CORES)))
    out = np.concatenate(
        [np.asarray(res.results[i]["o"]).astype(np.float32) for i in range(N_CORES)],
        axis=0,
    )
    out += orow[None, :]
    out += x_flat
    return out.reshape(B, S, E).astype(np.float32)
